# revision 9
# baseline (speedup 1.0000x reference)
"""DFFN Trainium2 kernel for nn_DFFN_81535659147929.

Pipeline: project_in (1x1 conv, 64->340) -> per-8x8-patch rFFT2 * learned
filter -> irFFT2 -> depthwise 3x3 conv -> GELU gate -> project_out (170->64).

Key algebra: the per-patch rFFT2*w->irFFT2 step is, per channel c, a linear
map M_c on the 64 patch pixels, and all M_c are simultaneously diagonalized
by the (channel-independent) orthonormal real 2D-DFT basis C:
M_c = C^T diag(lam_c) C.  So the whole FFT stage becomes two shared-weight
matmuls around a per-(channel,freq) scale.

Device pipeline per core (data-parallel shard: image b=core//2, row half
core%2, with an 8-row patch-aligned halo):
  Phase 1, per 2-patch block (128 pixels on partitions):
    A:  psumA[pix,ch] = x_block^T @ w_inT        (project_in, K=64)
    C:  psumB[frq,ch] = CC @ sA                  (forward transform)
    lam: sB = psumB * lam_tile                   (DVE)
    E:  psumZ[ch,pix] = sB_chunk^T @ CCrhs      (inverse transform, lands
        channel-major -> spatial z bands, DMA'd to a DRAM bounce buffer)
  Phase 2, per 16-row band (z re-read with 1-row halo):
    dwconv 3x3 as 9 shifted diag-matmuls per chunk accumulating in PSUM,
    GELU gate (ACT+DVE), project_out (K=170 over two chunks), DMA out.

Channel column order is permuted so the gate pairs (c, c+170) land on the
same PSUM partitions: cols = [x1a(128) | x2a(128) | x1b(42) | 0pad(22) |
x2b(42)]; the x2b block is placed at PE columns 64..105 via the inverse-
transform lhsT so the depthwise row-tile matmul at tile_position (64,0)
realigns it to partitions 0..41 for the gate.
"""

import os
import numpy as np
import ml_dtypes

import concourse.bass as bass
import concourse.mybir as mybir
from concourse import bacc, tile
from concourse.bass_utils import run_bass_kernel_spmd

BF16 = mybir.dt.bfloat16
F32 = mybir.dt.float32

DIM = 64
HIDDEN = 170
C2 = 340
P = 8
B, H, W = 4, 256, 256
N_CORES = 8
ROWS = H // 2          # 128 output rows per shard
HALO = P
NCOL = 362             # permuted channel columns incl 22-col zero pad
CH_E0 = 256            # chunk-E column base (x1b 42 | pad 22 | x2b 42)

_cache = {}


# ----------------------------------------------------------------- host math

def _build_basis():
    rows = []
    seen = set()
    p1, p2 = np.meshgrid(np.arange(P), np.arange(P), indexing="ij")
    for u in range(P):
        for v in range(P):
            if (u, v) in seen:
                continue
            nu, nv = (-u) % P, (-v) % P
            th = 2 * np.pi * (u * p1 + v * p2) / P
            if (nu, nv) == (u, v):
                rows.append((np.cos(th) / 8.0).ravel())
            else:
                seen.add((nu, nv))
                rows.append((np.sqrt(2) / 8.0) * np.cos(th).ravel())
                rows.append((np.sqrt(2) / 8.0) * np.sin(th).ravel())
            seen.add((u, v))
    C = np.array(rows, dtype=np.float64)
    return C


def _lam_for(fft_w, C):
    basis = C.reshape(64, P, P)
    F = np.fft.rfft2(basis)
    w = fft_w.reshape(C2, 1, P, P // 2 + 1).astype(np.float64)
    r = np.fft.irfft2(F[None] * w, s=(P, P))
    return np.einsum('kpq,ckpq->ck', basis, r)      # [C2, 64]


def _perm_cols():
    """col -> c2 channel (or -1 for pad)."""
    cols = np.full(NCOL, -1, np.int64)
    cols[0:128] = np.arange(0, 128)          # x1a
    cols[128:256] = np.arange(170, 298)      # x2a
    cols[256:298] = np.arange(128, 170)      # x1b
    cols[320:362] = np.arange(298, 340)      # x2b
    return cols


def _pix_maps():
    """CClhsT [128,128] (k,phi) and CCrhs [128,128] (phi,k) for 2-patch
    blocks; pixel k = p1*16+pc2*8+p2, freq phi = pc2*64+f."""
    C = _build_basis()
    CCrhs = np.zeros((128, 128))
    for pc2 in range(2):
        for f in range(64):
            for p1 in range(P):
                for p2 in range(P):
                    k = p1 * 16 + pc2 * 8 + p2
                    CCrhs[pc2 * 64 + f, k] = C[f, p1 * 8 + p2]
    return CCrhs.T.copy(), CCrhs, C


def _prep_weights(w_in, w_dw, fft_w, w_out):
    CClhsT, CCrhs, C = _pix_maps()
    lam = _lam_for(fft_w, C)                        # [340, 64]
    cols = _perm_cols()
    valid = cols >= 0

    w_inT = np.zeros((64, NCOL))
    w_inT[:, valid] = w_in.T[:, cols[valid]]

    lam_t = np.zeros((128, NCOL))
    lam_sel = np.zeros((NCOL, 64))
    lam_sel[valid] = lam[cols[valid]]
    lam_t[:] = np.tile(lam_sel.T, (2, 1))[:128]     # row phi -> lam[col, phi%64]

    dw = w_dw.reshape(C2, 9)
    # dd layout: [128, 9*128 (A) + 9*128 (B) + 9*42 (E)]
    dd = np.zeros((128, 9 * 128 + 9 * 128 + 9 * 42))
    for t in range(9):
        for p in range(128):
            dd[p, t * 128 + p] = dw[cols[p], t]                       # A
            dd[p, 9 * 128 + t * 128 + p] = dw[cols[128 + p], t]      # B
        for p in range(42):
            dd[p, 18 * 128 + t * 42 + p] = dw[cols[256 + p], t]      # E x1b
            dd[64 + p, 18 * 128 + t * 42 + p] = dw[cols[320 + p], t]  # E x2b
    # wo: [128, 128]: cols 0..63 = woA (gate-ch 0..127), 64..127 = woE rows 0..41
    wo = np.zeros((128, 128))
    wo[:, 0:64] = w_out.T[0:128]
    wo[0:42, 64:128] = w_out.T[128:170]

    bf = ml_dtypes.bfloat16
    return {
        "w_inT": w_inT.astype(bf),
        "cclhsT": CClhsT.astype(bf),
        "ccrhs": CCrhs.astype(bf),
        "lam_t": lam_t.astype(np.float32),
        "dd": dd.astype(bf),
        "wo": wo.astype(bf),
    }


# ---------------------------------------------------------------- bass build

def build_nc(rows=ROWS):
    """Build the per-core SPMD program. rows = output rows per shard."""
    rh = rows + 2 * HALO                 # z field rows incl halo
    npr = rh // P                        # patch rows
    nband = rows // 16                   # 16-row output bands
    assert rows % 16 == 0

    nc = bacc.Bacc("TRN2", target_bir_lowering=False, debug=False,
                   num_devices=N_CORES)
    x_d = nc.dram_tensor("x", [DIM, rh * W], BF16, kind="ExternalInput")
    winT_d = nc.dram_tensor("w_inT", [64, NCOL], BF16, kind="ExternalInput")
    cclhsT_d = nc.dram_tensor("cclhsT", [128, 128], BF16, kind="ExternalInput")
    ccrhs_d = nc.dram_tensor("ccrhs", [128, 128], BF16, kind="ExternalInput")
    lam_d = nc.dram_tensor("lam_t", [128, NCOL], F32, kind="ExternalInput")
    dd_d = nc.dram_tensor("dd", [128, 9 * 128 + 9 * 128 + 9 * 42], BF16,
                          kind="ExternalInput")
    wo_d = nc.dram_tensor("wo", [128, 128], BF16, kind="ExternalInput")
    out_d = nc.dram_tensor("out", [DIM, rows * W], F32, kind="ExternalOutput")

    zA_d = nc.dram_tensor("zbufA", [128, rh * W], BF16)
    zB_d = nc.dram_tensor("zbufB", [128, rh * W], BF16)
    zE_d = nc.dram_tensor("zbufE", [128, rh * W], BF16)

    G = mybir.ActivationFunctionType.Gelu

    with tile.TileContext(nc) as tc:
        with tc.tile_pool(name="consts", bufs=1) as cpool:
            w_inT = cpool.tile([64, NCOL], BF16)
            nc.sync.dma_start(out=w_inT[:], in_=winT_d[:])
            cclhsT = cpool.tile([128, 128], BF16)
            nc.sync.dma_start(out=cclhsT[:], in_=cclhsT_d[:])
            ccrhs = cpool.tile([128, 128], BF16)
            nc.sync.dma_start(out=ccrhs[:], in_=ccrhs_d[:])
            lam_t = cpool.tile([128, NCOL], F32)
            nc.sync.dma_start(out=lam_t[:], in_=lam_d[:])
            dd = cpool.tile([128, 9 * 128 + 9 * 128 + 9 * 42], BF16)
            nc.sync.dma_start(out=dd[:], in_=dd_d[:])
            wo = cpool.tile([128, 128], BF16)
            nc.sync.dma_start(out=wo[:], in_=wo_d[:])

            # ---------------- phase 1: x -> z (freq filter), bounce to DRAM
            with (
                tc.tile_pool(name="p1x", bufs=1) as xpool,
                tc.tile_pool(name="p1s", bufs=3) as spool,
                tc.tile_pool(name="p1z", bufs=2) as zpool,
                tc.tile_pool(name="p1ps", bufs=2, space="PSUM") as pspool,
                tc.tile_pool(name="p1pz", bufs=1, space="PSUM") as pzpool,
            ):
                # x arrives host-patchified: [64, (pr, pcp, p1, pc2, p2)],
                # so each 2-patch block is a contiguous 128-col lhsT slice.
                x_sb = xpool.tile([64, rh * W], BF16)
                nc.sync.dma_start(out=x_sb[:], in_=x_d[:])

                for band in range((npr + 1) // 2):
                    prs = [2 * band + d for d in range(2) if 2 * band + d < npr]
                    zbA = zpool.tile([128, len(prs) * 8 * W], BF16, tag="zbA")
                    zbB = zpool.tile([128, len(prs) * 8 * W], BF16, tag="zbB")
                    zbE = zpool.tile([128, len(prs) * 8 * W], BF16, tag="zbE")
                    for bi, pr in enumerate(prs):
                        for pcp in range(16):
                            psA = pspool.tile([128, NCOL], F32, tag="psA")
                            xblk = x_sb[:, (pr * 16 + pcp) * 128:
                                        (pr * 16 + pcp) * 128 + 128]
                            nc.tensor.matmul(psA[:], xblk, w_inT[:],
                                             start=True, stop=True)
                            sA = spool.tile([128, NCOL], BF16, tag="sA")
                            nc.scalar.copy(sA[:], psA[:])
                            psB = pspool.tile([128, NCOL], F32, tag="psB")
                            nc.tensor.matmul(psB[:], cclhsT[:], sA[:],
                                             start=True, stop=True)
                            sB = spool.tile([128, NCOL], BF16, tag="sB")
                            nc.vector.tensor_mul(sB[:], psB[:], lam_t[:])
                            for name, zb, c0, m in (
                                    ("A", zbA, 0, 128), ("B", zbB, 128, 128),
                                    ("E", zbE, CH_E0, 106)):
                                psZ = pzpool.tile([128, 128], F32,
                                                  tag=f"psZ{name}")
                                nc.tensor.matmul(
                                    psZ[0:m, :], sB[:, c0:c0 + m], ccrhs[:],
                                    start=True, stop=True)
                                zw = zb[:].rearrange(
                                    "c (r p1 pcp pc2 p2) -> c r pcp p1 pc2 p2",
                                    r=len(prs), p1=8, pcp=16, pc2=2, p2=8)
                                nc.scalar.copy(zw[0:m, bi, pcp], psZ[0:m, :])
                    r0 = prs[0] * 8 * W
                    rn = len(prs) * 8 * W
                    nc.sync.dma_start(out=zA_d[:, r0:r0 + rn], in_=zbA[:])
                    nc.sync.dma_start(out=zB_d[:, r0:r0 + rn], in_=zbB[:])
                    nc.sync.dma_start(out=zE_d[:, r0:r0 + rn], in_=zbE[:])

            # ---------------- phase 2: dwconv + gate + project_out
            with (
                tc.tile_pool(name="p2z", bufs=2) as zrpool,
                tc.tile_pool(name="p2g", bufs=2) as gpool,
                tc.tile_pool(name="p2o", bufs=2) as opool,
                tc.tile_pool(name="p2pd", bufs=1, space="PSUM") as pdpool,
                tc.tile_pool(name="p2po", bufs=2, space="PSUM") as popool,
            ):
                # taps: (dy, dx) with band-local row/col windows
                shifts = []
                for dy in range(3):
                    for dx in range(3):
                        wi0, wo0, wn = ((0, 1, 255) if dx == 0 else
                                        (0, 0, 256) if dx == 1 else (1, 0, 255))
                        shifts.append((dy * 3 + dx, dy, wi0, wo0, wn))

                for band in range(nband):
                    zr0 = (HALO + 16 * band - 1) * W
                    zrA = zrpool.tile([128, 18 * W], BF16, tag="zrA")
                    nc.sync.dma_start(out=zrA[:], in_=zA_d[:, zr0:zr0 + 18 * W])
                    zrB = zrpool.tile([128, 18 * W], BF16, tag="zrB")
                    nc.sync.dma_start(out=zrB[:], in_=zB_d[:, zr0:zr0 + 18 * W])
                    zrE = zrpool.tile([128, 18 * W], BF16, tag="zrE")
                    nc.sync.dma_start(out=zrE[:], in_=zE_d[:, zr0:zr0 + 18 * W])
                    obnd = opool.tile([64, 16 * W], F32, tag="oband")

                    for ct in range(8):          # 2-row column tiles
                        j0 = 2 * ct
                        dps = {}
                        for name, zr, ddof, pb, m in (
                                ("A", zrA, 0, 0, 128),
                                ("B", zrB, 9 * 128, 0, 128),
                                ("E1", zrE, 18 * 128, 0, 42),
                                ("E2", zrE, 18 * 128, 64, 42)):
                            dp = pdpool.tile([m, 512], F32, tag=f"d{name}")
                            dpr = dp[:].rearrange("c (r w) -> c r w", r=2, w=W)
                            zrr = zr[:].rearrange("c (r w) -> c r w", r=18, w=W)
                            dw_w = 128 if m == 128 else 42
                            for ti, (t, dy, wi0, wo0, wn) in enumerate(shifts):
                                lhs = dd[pb:pb + m,
                                         ddof + t * dw_w:ddof + t * dw_w + m]
                                nc.tensor.matmul(
                                    dpr[:, :, wo0:wo0 + wn],
                                    lhs,
                                    zrr[pb:pb + m, j0 + dy:j0 + dy + 2,
                                        wi0:wi0 + wn],
                                    start=(ti == 0), stop=(ti == 8),
                                    tile_position=(pb, 0))
                            dps[name] = dp

                        gelA = gpool.tile([128, 512], BF16, tag="gelA")
                        nc.scalar.activation(gelA[:], dps["A"][:], G)
                        gA = gpool.tile([128, 512], BF16, tag="gA")
                        nc.vector.tensor_mul(gA[:], gelA[:], dps["B"][:])
                        gelE = gpool.tile([42, 512], BF16, tag="gelE")
                        nc.scalar.activation(gelE[:], dps["E1"][:], G)
                        gE = gpool.tile([42, 512], BF16, tag="gE")
                        nc.vector.tensor_mul(gE[:], gelE[:], dps["E2"][:])

                        po = popool.tile([64, 512], F32, tag="po")
                        nc.tensor.matmul(po[:], wo[:, 0:64], gA[:],
                                         start=True, stop=False)
                        nc.tensor.matmul(po[:], wo[0:42, 64:128], gE[:],
                                         start=False, stop=True)
                        nc.scalar.copy(obnd[:, ct * 512:(ct + 1) * 512], po[:])

                    nc.sync.dma_start(
                        out=out_d[:, band * 16 * W:(band + 1) * 16 * W],
                        in_=obnd[:])

    nc.compile()
    return nc


# ----------------------------------------------------------------- interface

def _get_program(rows=ROWS):
    key = ("nc", rows)
    if key not in _cache:
        _cache[key] = build_nc(rows)
    return _cache[key]


def _patchify(xs):
    """[64, rh, 256] -> [64, (pr, pcp, p1, pc2, p2)] flat bf16."""
    rh = xs.shape[1]
    xp = xs.reshape(DIM, rh // 8, 8, 16, 2, 8).transpose(0, 1, 3, 2, 4, 5)
    return np.ascontiguousarray(xp).reshape(DIM, rh * W).astype(
        ml_dtypes.bfloat16)


def _shard_x(x, rows=ROWS):
    """Per-core halo-padded, patchified bf16 shards."""
    rh = rows + 2 * HALO
    shards = []
    for c in range(N_CORES):
        b, hh = divmod(c, 2)
        r0 = hh * rows
        xs = np.zeros((DIM, rh, W), np.float32)
        lo, hi = r0 - HALO, r0 + rows + HALO
        slo, shi = max(lo, 0), min(hi, x.shape[2])
        xs[:, slo - lo:shi - lo] = x[b, :, slo:shi]
        shards.append(_patchify(xs))
    return shards


def _run(x, w_in, w_dw, fft_w, w_out, trace=False):
    nc = _get_program()
    wts = _prep_weights(np.asarray(w_in, np.float32),
                        np.asarray(w_dw, np.float32).reshape(C2, 3, 3),
                        np.asarray(fft_w, np.float32),
                        np.asarray(w_out, np.float32))
    shards = _shard_x(np.asarray(x, np.float32))
    in_maps = [{"x": s, **wts} for s in shards]
    res = run_bass_kernel_spmd(nc, in_maps, core_ids=list(range(N_CORES)),
                               trace=trace)
    out = np.zeros((B, DIM, H, W), np.float32)
    for c in range(N_CORES):
        b, hh = divmod(c, 2)
        out[b, :, hh * ROWS:(hh + 1) * ROWS] = (
            res.results[c]["out"].reshape(DIM, ROWS, W))
    return out, res.exec_time_ns


def kernel(x, w_in, w_dw, fft_w, w_out):
    out, _ = _run(x, w_in, w_dw, fft_w, w_out, trace=False)
    return out
